# revision 11
# baseline (speedup 1.0000x reference)
"""Trainium2 Bass kernel for nn_Decoder (LAS-style attention LSTM decoder).

Strategy: data-parallel over batch N=128 across 8 NeuronCores (16/core),
batch elements assigned to cores by lens-sorted snake ordering so每core's
attention work (sum of lens) is balanced. One SPMD program; per-core
raggedness is handled by padding every core's slot-s sequence to the max
tile count of that slot across cores (zero-padded values/ones columns make
the extra work a numerical no-op).

Per step (all on device):
  gates1 = onehot(text_t) @ M + ctx @ W_c^T + h1 @ W_hh1^T   (psum accumulate;
           M = emb @ W_e^T with a bias row, computed in a device prologue)
  LSTM pointwise via tanh only (sigmoid(x) = 0.5*tanh(x/2)+0.5; keeps ACT in
           the exp/tanh table set -> no table reloads)
  gates2 = b2 + h1 @ W_ih2^T + h2 @ W_hh2^T
  scores: per (slot, t-tile) matmul with the key tile as the stationary
           operand -> scores land t-major in one PSUM bank; single Exp over
           [128, PAIRS]; no max subtraction (|scores| < ~6).
  ctx:    per (slot, t-tile) accumulating matmul, e-column as stationary;
           values carry an extra ones-column so Z (softmax denom) falls out
           of the same matmuls; tile_position spreads 16 slots over 4
           column groups. Similar-length slot chains are paired across the
           two ctx PSUM banks and their matmuls interleaved, so two chains
           stream through distinct PE column groups concurrently (safe under
           the whole-bank has_written clear: within a bank chains stay
           sequential). Gate blocks are pre-permuted [i,f,o,g] host-side so
           the three sigmoid-path activations fuse into one ACT + one DVE op
           per LSTM cell.
  pred = [h2, ctx] @ W_mos^T + b_mos accumulated into out staging, DMA'd out.

fp16 is used for the gates matmul operands + key/scores operands (validated
~2e-4 absmax vs fp32 reference); values/e/ctx/states/softmax stay fp32.
"""

import math
import os
import sys
from contextlib import ExitStack

import numpy as np

sys.path.insert(0, "/opt/trn_rl_repo")

T_FULL, N_FULL, L_STEPS = 2000, 128, 250
H, KD, VD, AD = 512, 128, 128, 64
NB = 16          # batch per core
N_CORES = 8
TT = 128         # t-tile size


def _plan(lens):
    """Assign batch indices to (core, slot) and compute uniform slot tiling."""
    order = np.argsort(lens)[::-1]          # descending
    perm = [[None] * NB for _ in range(N_CORES)]
    slot_of_rank = []
    for r, idx in enumerate(order):
        blk, pos = divmod(r, N_CORES)
        c = pos if blk % 2 == 0 else N_CORES - 1 - pos
        s = blk
        perm[c][s] = int(idx)
    # slot tile counts: max over cores of ceil(lens/TT) for that slot
    F = []
    for s in range(NB):
        mx = max(int(math.ceil(lens[perm[c][s]] / TT)) for c in range(N_CORES))
        F.append(max(mx, 1))
    cum = np.concatenate([[0], np.cumsum(F)]).astype(int)
    pairs = int(cum[-1])
    return perm, F, cum, pairs


def _build_program(F, cum, PAIRS, L=L_STEPS, REPS=1):
    import concourse.bass as bass
    import concourse.mybir as mybir
    import concourse.tile as tile
    from concourse import bacc

    f32 = mybir.dt.float32
    f16 = mybir.dt.float16

    nc = bacc.Bacc(None, target_bir_lowering=False)

    # ---------------- DRAM I/O ----------------
    keyT_d = nc.dram_tensor("keyT", [128, PAIRS * TT], f16, kind="ExternalInput")
    valsT_d = nc.dram_tensor("valsT", [128, PAIRS * (VD + 1)], f16, kind="ExternalInput")
    wg1_d = nc.dram_tensor("Wg1T", [128, 5 * 4 * H], f16, kind="ExternalInput")
    wg2_d = nc.dram_tensor("Wg2T", [128, 5 * 4 * KD], f16, kind="ExternalInput")
    oneh_d = nc.dram_tensor("onehotT", [AD + 1, NB * L_STEPS], f16, kind="ExternalInput")
    embT_d = nc.dram_tensor("embT", [128, 4 * AD], f32, kind="ExternalInput")
    weT_d = nc.dram_tensor("WeT", [H, 4 * H], f32, kind="ExternalInput")
    bih1_d = nc.dram_tensor("b_ih1", [1, 4 * H], f32, kind="ExternalInput")
    bhh1_d = nc.dram_tensor("b_hh1", [1, 4 * H], f32, kind="ExternalInput")
    bih2_d = nc.dram_tensor("b_ih2", [1, 4 * KD], f32, kind="ExternalInput")
    bhh2_d = nc.dram_tensor("b_hh2", [1, 4 * KD], f32, kind="ExternalInput")
    wmos_d = nc.dram_tensor("WmosT2", [128, 2 * AD], f16, kind="ExternalInput")
    bmos_d = nc.dram_tensor("b_mos_col", [AD, 1], f32, kind="ExternalInput")
    eye_d = nc.dram_tensor("eye16", [16, 16], f32, kind="ExternalInput")
    ctx0_d = nc.dram_tensor("ctx0T", [128, NB], f16, kind="ExternalInput")
    out_d = nc.dram_tensor("out", [AD, NB * L_STEPS], f32, kind="ExternalOutput")

    GCH = 512                      # gates1 chunk
    OUT_GRP = 10 if L % 10 == 0 else 1   # steps per output DMA

    with tile.TileContext(nc) as tc, ExitStack() as ctx:
        const = ctx.enter_context(tc.tile_pool(name="const", bufs=1))
        stage = ctx.enter_context(tc.tile_pool(name="stage", bufs=2))
        psbig = ctx.enter_context(tc.tile_pool(name="psbig", bufs=1, space="PSUM"))
        pssp = ctx.enter_context(tc.tile_pool(name="pssp", bufs=1, space="PSUM"))
        pscps = ctx.enter_context(tc.tile_pool(name="pscps", bufs=1, space="PSUM"))
        pssm = ctx.enter_context(tc.tile_pool(name="pssm", bufs=1, space="PSUM"))
        work = ctx.enter_context(tc.tile_pool(name="work", bufs=1))
        outp = ctx.enter_context(tc.tile_pool(name="outp", bufs=2))

        # ---------------- resident SBUF tensors ----------------
        keyT = const.tile([128, PAIRS * TT], f16, tag="keyT")
        nc.sync.dma_start(keyT[:], keyT_d[:])
        valsT = const.tile([128, PAIRS * (VD + 1)], f16, tag="valsT")
        nc.sync.dma_start(valsT[:], valsT_d[:])
        wg1 = const.tile([128, 5 * 4 * H], f16, tag="wg1")
        nc.sync.dma_start(wg1[:], wg1_d[:])
        wg2 = const.tile([128, 5 * 4 * KD], f16, tag="wg2")
        nc.sync.dma_start(wg2[:], wg2_d[:])
        oneh = const.tile([AD + 1, NB * L_STEPS], f16, tag="oneh")
        nc.sync.dma_start(oneh[:], oneh_d[:])
        wmos = const.tile([128, 2 * AD], f16, tag="wmos")
        nc.sync.dma_start(wmos[:], wmos_d[:])
        bmos = const.tile([AD, 1], f32, tag="bmos")
        nc.sync.dma_start(bmos[:], bmos_d[:])
        eye16 = const.tile([16, 16], f32, tag="eye16")
        nc.sync.dma_start(eye16[:], eye_d[:])
        ones_row16 = const.tile([1, 16], f16, tag="ones_row16")
        nc.vector.memset(ones_row16[:], 1.0)
        # b2 = b_ih2 + b_hh2 summed on the PE (TT instrs only fit one sync wait)
        b2row = const.tile([1, 4 * KD], f16, tag="b2row")
        bi2 = stage.tile([1, 4 * KD], f32, tag="westg")
        nc.sync.dma_start(bi2[:], bih2_d[:])
        bh2 = stage.tile([1, 4 * KD], f32, tag="westg")
        nc.sync.dma_start(bh2[:], bhh2_d[:])
        one11 = const.tile([1, 1], f16, tag="one11")
        nc.vector.memset(one11[:], 1.0)
        one11f = const.tile([1, 1], f32, tag="one11f")
        nc.vector.memset(one11f[:], 1.0)
        b2ps = pssm.tile([1, 4 * KD], f32, tag="sm")
        nc.tensor.matmul(b2ps[:], lhsT=one11f[:], rhs=bi2[:], start=True, stop=False)
        nc.tensor.matmul(b2ps[:], lhsT=one11f[:], rhs=bh2[:], start=False, stop=True)
        nc.vector.tensor_copy(b2row[:], b2ps[:])

        # ---------------- prologue: M = emb @ W_e^T (+ bias row) ----------------
        M = const.tile([AD + 1, 4 * H], f16, tag="M")
        embT = stage.tile([128, 4 * AD], f32, tag="embT")   # 4 k-tiles of [128, 64]
        nc.sync.dma_start(embT[:], embT_d[:])
        mps = psbig.tile([AD + 1, 4 * H], f32, tag="big")
        for kt in range(4):
            wstg = stage.tile([128, 4 * H], f32, tag="westg")
            nc.sync.dma_start(wstg[:], weT_d[kt * 128:(kt + 1) * 128, :])
            for ch in range(4):
                nc.tensor.matmul(
                    mps[0:AD, ch * GCH:(ch + 1) * GCH],
                    lhsT=embT[:, kt * AD:(kt + 1) * AD],
                    rhs=wstg[:, ch * GCH:(ch + 1) * GCH],
                    start=(kt == 0), stop=(kt == 3),
                )
        # bias row 64 = b_ih1 + b_hh1, accumulated on the PE
        bi1 = stage.tile([1, 4 * H], f32, tag="westg")
        nc.sync.dma_start(bi1[:], bih1_d[:])
        bh1 = stage.tile([1, 4 * H], f32, tag="westg")
        nc.sync.dma_start(bh1[:], bhh1_d[:])
        for ch in range(4):
            cs = slice(ch * GCH, (ch + 1) * GCH)
            nc.tensor.matmul(mps[AD:AD + 1, cs], lhsT=one11f[:], rhs=bi1[0:1, cs],
                             start=True, stop=False, tile_position=(0, 64))
            nc.tensor.matmul(mps[AD:AD + 1, cs], lhsT=one11f[:], rhs=bh1[0:1, cs],
                             start=False, stop=True, tile_position=(0, 64))
        nc.vector.tensor_copy(M[:], mps[:])

        # ---------------- persistent state ----------------
        c1 = const.tile([NB, H], f32, tag="c1")
        nc.vector.memset(c1[:], 0.0)
        c2 = const.tile([NB, KD], f32, tag="c2")
        nc.vector.memset(c2[:], 0.0)
        h1T = const.tile([128, 4 * NB], f16, tag="h1T")
        nc.vector.memset(h1T[:], 0.0)
        h2T = const.tile([128, NB], f16, tag="h2T")
        nc.vector.memset(h2T[:], 0.0)
        ctxT = const.tile([128, NB], f16, tag="ctxT")
        nc.sync.dma_start(ctxT[:], ctx0_d[:])

        e_sb = const.tile([128, PAIRS], f16, tag="e_sb")
        cprow = const.tile([128, 2 * 512], f32, tag="cprow")
        zrow = const.tile([128, 4], f32, tag="zrow")

        # ctx psum slot columns: q0,q1 contiguous in bank 0; q2,q3 in bank 1
        # (gap-free spans so the row drains only read written psum bytes)
        CTX_OFF = [0, 129, 512, 641]
        # Pairwise-interleaved ctx chains: slot 2k -> a bank-0 cell, slot 2k+1
        # -> a bank-1 cell in a different PE column group, and their matmuls
        # alternate so the two chains stream concurrently (distinct col groups
        # + distinct PSUM banks keeps the has_written bank-clear safe).
        # cell_of[s] = (q, j): column offset CTX_OFF[q], psum row 32*j.
        cell_of = {}
        for k in range(NB // 2):
            cell_of[2 * k] = (k // 4, k % 4)
            cell_of[2 * k + 1] = (2 + k // 4, (k + 1) % 4)

        out_stage = None

        for t in [tt for _ in range(REPS) for tt in range(L)]:
            # ---------- gates1 ----------
            # ctx-dependent matmul last so the E- and h1-parts can issue
            # while the previous step's attention is still draining
            g1 = psbig.tile([NB, 4 * H], f32, tag="big")
            for ch in range(4):
                cs = slice(ch * GCH, (ch + 1) * GCH)
                nc.tensor.matmul(
                    g1[:, cs], lhsT=oneh[:, t * NB:(t + 1) * NB],
                    rhs=M[:, cs], start=True, stop=False)
                for i in range(4):
                    nc.tensor.matmul(
                        g1[:, cs], lhsT=h1T[:, i * NB:(i + 1) * NB],
                        rhs=wg1[:, (1 + i) * 4 * H + ch * GCH:(1 + i) * 4 * H + (ch + 1) * GCH],
                        start=False, stop=False)
                nc.tensor.matmul(
                    g1[:, cs], lhsT=ctxT[:], rhs=wg1[:, cs], start=False, stop=True)
            # ---------- pointwise 1 (gates laid out [i,f,o,g]) ----------
            t_ifo = work.tile([NB, 3 * H], f32, tag="t_ifo")
            nc.scalar.activation(t_ifo[:], g1[:, 0:3 * H], mybir.ActivationFunctionType.Tanh, scale=0.5)
            t_g = work.tile([NB, H], f32, tag="t_g")
            nc.scalar.activation(t_g[:], g1[:, 3 * H:4 * H], mybir.ActivationFunctionType.Tanh)
            nc.vector.tensor_scalar(out=t_ifo[:], in0=t_ifo[:], scalar1=0.5, scalar2=0.5,
                                    op0=mybir.AluOpType.mult, op1=mybir.AluOpType.add)
            nc.vector.tensor_tensor(out=c1[:], in0=c1[:], in1=t_ifo[:, H:2 * H], op=mybir.AluOpType.mult)
            tmp = work.tile([NB, H], f32, tag="tmp")
            nc.vector.tensor_tensor(out=tmp[:], in0=t_ifo[:, 0:H], in1=t_g[:], op=mybir.AluOpType.mult)
            nc.vector.tensor_tensor(out=c1[:], in0=c1[:], in1=tmp[:], op=mybir.AluOpType.add)
            t_c = work.tile([NB, H], f32, tag="tmp")
            nc.scalar.activation(t_c[:], c1[:], mybir.ActivationFunctionType.Tanh)
            h1 = work.tile([NB, H], f32, tag="h1")
            nc.vector.tensor_tensor(out=h1[:], in0=t_ifo[:, 2 * H:3 * H], in1=t_c[:], op=mybir.AluOpType.mult)
            # h1T (fp16)
            h1tp = pssm.tile([128, 4 * NB], f32, tag="sm")
            for i in range(4):
                nc.tensor.transpose(h1tp[:, i * NB:(i + 1) * NB], h1[:, i * 128:(i + 1) * 128], eye16[:])
            nc.vector.tensor_copy(h1T[:], h1tp[:])
            # ---------- gates2 ----------
            g2 = pssm.tile([NB, 4 * KD], f32, tag="sm")
            nc.tensor.matmul(g2[:], lhsT=ones_row16[:], rhs=b2row[:], start=True, stop=False)
            for i in range(4):
                nc.tensor.matmul(g2[:], lhsT=h1T[:, i * NB:(i + 1) * NB],
                                 rhs=wg2[:, i * 512:(i + 1) * 512],
                                 start=False, stop=False)
            nc.tensor.matmul(g2[:], lhsT=h2T[:], rhs=wg2[:, 4 * 512:5 * 512], start=False, stop=True)
            # ---------- pointwise 2 (gates laid out [i,f,o,g]) ----------
            t_ifo2 = work.tile([NB, 3 * KD], f32, tag="t_ifo2")
            nc.scalar.activation(t_ifo2[:], g2[:, 0:3 * KD], mybir.ActivationFunctionType.Tanh, scale=0.5)
            t_g2 = work.tile([NB, KD], f32, tag="t_g2")
            nc.scalar.activation(t_g2[:], g2[:, 3 * KD:4 * KD], mybir.ActivationFunctionType.Tanh)
            nc.vector.tensor_scalar(out=t_ifo2[:], in0=t_ifo2[:], scalar1=0.5, scalar2=0.5,
                                    op0=mybir.AluOpType.mult, op1=mybir.AluOpType.add)
            nc.vector.tensor_tensor(out=c2[:], in0=c2[:], in1=t_ifo2[:, KD:2 * KD], op=mybir.AluOpType.mult)
            tmp2 = work.tile([NB, KD], f32, tag="tmp2")
            nc.vector.tensor_tensor(out=tmp2[:], in0=t_ifo2[:, 0:KD], in1=t_g2[:], op=mybir.AluOpType.mult)
            nc.vector.tensor_tensor(out=c2[:], in0=c2[:], in1=tmp2[:], op=mybir.AluOpType.add)
            t_c2 = work.tile([NB, KD], f32, tag="tmp2")
            nc.scalar.activation(t_c2[:], c2[:], mybir.ActivationFunctionType.Tanh)
            h2 = work.tile([NB, KD], f32, tag="h2")
            nc.vector.tensor_tensor(out=h2[:], in0=t_ifo2[:, 2 * KD:3 * KD], in1=t_c2[:], op=mybir.AluOpType.mult)
            h2tp = pssm.tile([128, NB], f32, tag="sm")
            nc.tensor.transpose(h2tp[:], h2[:], eye16[:])
            nc.vector.tensor_copy(h2T[:], h2tp[:])
            # ---------- scores (t-major) + exp ----------
            sp = pssp.tile([128, PAIRS], f32, tag="sp")
            for s in range(NB):
                for f in range(F[s]):
                    p = cum[s] + f
                    nc.tensor.matmul(
                        sp[:, p:p + 1],
                        lhsT=keyT[:, p * TT:(p + 1) * TT],
                        rhs=h2T[:, s:s + 1], start=True, stop=True)
            nc.scalar.activation(e_sb[:], sp[:], mybir.ActivationFunctionType.Exp)
            # ---------- ctx (+Z via ones col) ----------
            cps = pscps.tile([128, 2 * GCH], f32, tag="cps")
            for k in range(NB // 2):
                sa, sb = 2 * k, 2 * k + 1
                for f in range(max(F[sa], F[sb])):
                    for s in (sa, sb):
                        if f >= F[s]:
                            continue
                        q, j = cell_of[s]
                        co = CTX_OFF[q]
                        p = cum[s] + f
                        nc.tensor.matmul(
                            cps[32 * j:32 * j + 1, co:co + VD + 1],
                            lhsT=e_sb[:, p:p + 1],
                            rhs=valsT[:, p * (VD + 1):(p + 1) * (VD + 1)],
                            start=(f == 0), stop=(f == F[s] - 1),
                            tile_position=(0, 32 * j))
            # drain ctx rows to SBUF (strided-partition APs illegal on DVE/ACT;
            # use single-partition ops, alternating engines)
            for j in range(4):
                row = slice(32 * j, 32 * j + 1)
                if j % 2 == 0:
                    nc.vector.tensor_copy(cprow[row, 0:258], cps[row, 0:258])
                    nc.scalar.copy(cprow[row, 512:770], cps[row, 512:770])
                else:
                    nc.scalar.copy(cprow[row, 0:258], cps[row, 0:258])
                    nc.vector.tensor_copy(cprow[row, 512:770], cps[row, 512:770])
                nc.vector.reciprocal(zrow[row, 0:2], cprow[row, VD:VD + 130:129])
                nc.vector.reciprocal(zrow[row, 2:4], cprow[row, 512 + VD:512 + VD + 130:129])
            # ctx row-transposes, normalization folded into the moving operand
            ctp = pssm.tile([128, NB], f32, tag="sm")
            jbuckets = [[s for s in range(NB) if cell_of[s][1] == jj] for jj in range(4)]
            for s in [s for grp in zip(*jbuckets) for s in grp]:
                q, j = cell_of[s]
                row = slice(32 * j, 32 * j + 1)
                nc.tensor.matmul(
                    ctp[:, s:s + 1],
                    lhsT=cprow[row, CTX_OFF[q]:CTX_OFF[q] + VD],
                    rhs=zrow[row, q:q + 1],
                    start=True, stop=True, tile_position=(32 * j, 0))
            nc.vector.tensor_copy(ctxT[:], ctp[:])
            # ---------- MoS output ----------
            mps2 = pssm.tile([AD, NB], f32, tag="sm")
            nc.tensor.matmul(mps2[:], lhsT=wmos[:, 0:AD], rhs=h2T[:], start=True, stop=False)
            nc.tensor.matmul(mps2[:], lhsT=wmos[:, AD:2 * AD], rhs=ctxT[:], start=False, stop=True)
            if t % OUT_GRP == 0:
                out_stage = outp.tile([AD, OUT_GRP * NB], f32, tag="outs")
            nc.vector.tensor_scalar(
                out=out_stage[:, (t % OUT_GRP) * NB:(t % OUT_GRP + 1) * NB],
                in0=mps2[:], scalar1=bmos[:], scalar2=None, op0=mybir.AluOpType.add)
            if t % OUT_GRP == OUT_GRP - 1:
                nc.sync.dma_start(
                    out_d[:, (t - OUT_GRP + 1) * NB:(t + 1) * NB], out_stage[:])
    return nc


def _prep_inputs(inputs, perm, F, cum, PAIRS):
    """Build the 8 per-core input maps (host-side layout only)."""
    key = np.asarray(inputs["key"], np.float32)
    values = np.asarray(inputs["values"], np.float32)
    lens = np.asarray(inputs["lens"]).astype(np.int64)
    text = np.asarray(inputs["text"]).astype(np.int64)
    emb = np.asarray(inputs["emb"], np.float32)
    W_ih1 = np.asarray(inputs["W_ih1"], np.float32)
    W_hh1 = np.asarray(inputs["W_hh1"], np.float32)
    W_ih2 = np.asarray(inputs["W_ih2"], np.float32)
    W_hh2 = np.asarray(inputs["W_hh2"], np.float32)
    W_mos = np.asarray(inputs["W_mos"], np.float32)

    W_e = W_ih1[:, :H]
    W_c = W_ih1[:, H:]
    # permute gate blocks [i,f,g,o] -> [i,f,o,g] so sigmoid-path activations
    # (i,f,o) are contiguous and fuse into single ACT/DVE ops in the kernel
    pg1 = np.concatenate([np.arange(0, 2 * H), np.arange(3 * H, 4 * H),
                          np.arange(2 * H, 3 * H)])
    pg2 = np.concatenate([np.arange(0, 2 * KD), np.arange(3 * KD, 4 * KD),
                          np.arange(2 * KD, 3 * KD)])
    wg1 = np.concatenate([W_c.T[:, pg1]] + [W_hh1[:, i * 128:(i + 1) * 128].T[:, pg1]
                                            for i in range(4)],
                         axis=1).astype(np.float16)          # [128, 5*2048]
    wg2 = np.concatenate([W_ih2[:, i * 128:(i + 1) * 128].T[:, pg2] for i in range(4)]
                         + [W_hh2.T[:, pg2]], axis=1).astype(np.float16)  # [128, 5*512]
    wmos2 = np.concatenate([W_mos[:, :128].T, W_mos[:, 128:].T], axis=1).astype(np.float16)
    eye16 = np.eye(16, dtype=np.float32)

    shared = {
        "Wg1T": wg1, "Wg2T": wg2, "WmosT2": wmos2,
        "embT": np.concatenate([emb.T[i * 128:(i + 1) * 128] for i in range(4)],
                               axis=1).copy(),
        "WeT": W_e.T[:, pg1].copy(),
        "b_ih1": np.asarray(inputs["b_ih1"], np.float32)[None, pg1],
        "b_hh1": np.asarray(inputs["b_hh1"], np.float32)[None, pg1],
        "b_ih2": np.asarray(inputs["b_ih2"], np.float32)[None, pg2],
        "b_hh2": np.asarray(inputs["b_hh2"], np.float32)[None, pg2],
        "b_mos_col": np.asarray(inputs["b_mos"], np.float32)[:, None],
        "eye16": eye16,
    }

    in_maps = []
    for c in range(N_CORES):
        idxs = perm[c]
        keyT = np.zeros((128, PAIRS * TT), np.float16)
        valsT = np.zeros((128, PAIRS * (VD + 1)), np.float16)
        for s in range(NB):
            idx = idxs[s]
            ln = int(lens[idx])
            for f in range(F[s]):
                p = cum[s] + f
                t0, t1 = f * TT, min((f + 1) * TT, T_FULL)
                nrow = t1 - t0
                if nrow <= 0:
                    continue
                keyT[:, p * TT:p * TT + nrow] = key[t0:t1, idx, :].T
                nvalid = max(0, min(ln - t0, nrow))
                if nvalid > 0:
                    blk = valsT[:, p * (VD + 1):(p + 1) * (VD + 1)]
                    blk[0:nvalid, 0:VD] = values[t0:t0 + nvalid, idx, :]
                    blk[0:nvalid, VD] = 1.0
        # one-hot text, column order (t, s)
        oneh = np.zeros((AD + 1, NB * L_STEPS), np.float16)
        oneh[AD, :] = 1.0
        for s in range(NB):
            tx = text[idxs[s]]          # [250]
            cols = np.arange(L_STEPS) * NB + s
            valid = tx != 0
            oneh[tx[valid], cols[valid]] = 1.0
        ctx0T = values[0, idxs, :].T.astype(np.float16).copy()
        m = dict(shared)
        m.update({"keyT": keyT, "valsT": valsT, "onehotT": oneh, "ctx0T": ctx0T})
        in_maps.append(m)
    return in_maps


_CACHE = {}
LAST_RESULT = None


def kernel(**inputs):
    from concourse.bass_utils import run_bass_kernel_spmd

    lens = np.asarray(inputs["lens"]).astype(np.int64)
    perm, F, cum, PAIRS = _plan(lens)
    in_maps = _prep_inputs(inputs, perm, F, cum, PAIRS)

    sig = tuple(F)
    if sig not in _CACHE:
        nc = _build_program(F, cum, PAIRS)
        if not nc.is_finalized():
            nc.finalize()
        _CACHE[sig] = nc
    nc = _CACHE[sig]

    trace = os.environ.get("BASS_KERNEL_TRACE") == "1"
    res = run_bass_kernel_spmd(nc, in_maps, core_ids=list(range(N_CORES)),
                               trace=trace,
                               tmpdir=os.environ.get("BASS_KERNEL_TRACE_DIR"))
    global LAST_RESULT
    LAST_RESULT = res
    out = np.zeros((N_FULL, L_STEPS, AD), np.float32)
    for c in range(N_CORES):
        o = res.results[c]["out"]            # [64, 250*16] cols (t, s)
        o = o.reshape(AD, L_STEPS, NB)       # [a, t, s]
        for s in range(NB):
            out[perm[c][s]] = o[:, :, s].T
    return out


if __name__ == "__main__":
    d = np.load("/root/problem/inputs_cache.npz")
    out = kernel(**{k: d[k] for k in d.files})
    ref = np.load("/root/problem/ref_out.npy")
    err = np.abs(out - ref).max()
    print("absmax err:", err, "rel:", err / np.abs(ref).max())

